# revision 22
# baseline (speedup 1.0000x reference)
"""FlowNetC correlation kernel for Trainium2 (8 NeuronCores, batch-sharded).

out[b, d, y, x] = mean_c in1[b,c,y,x] * in2pad[b,c, y+dy, x+dx],
d = dyi*21 + dxi, dy = 2*dyi-20, dx = 2*dxi-20  (441 displacements).

Design (per core, 2 batch elements):
  Parity decomposition: dy, dx are even, so (y,x) only pairs with (y',x')
  of equal parity. Weight sets = 128 in1 columns: 8 same-parity rows
  (yi) x 16 same-parity columns (xi), stored contiguous in SBUF so each
  LDWEIGHTS is one contiguous 128-wide load. Moving operand = in2 band
  [128c, t, 36 u']: same-parity rows y' = 16g+yp+2t-20 clipped to
  [0,48) and x-positions u' = 16*xw + xi + dxi (x padded to 104 on
  host). Out-of-range displacements become host-side zeros; no padding
  rows, no memsets. 4 matmuls per set (2 cc chunks x 2 PSUM banks,
  N <= 504), 192 total, full 128 PSUM partitions, measured warm at
  2.4 GHz (a dummy-matmul burst at t=0 pre-warms the HAM clock gate).
  Per set DVE extracts PSUM bank 0 and ACT bank 1 in parallel (valid
  rows only, cast bf16) into a compact [128, 4320] staging buffer; one
  contiguous ~1.1 MB DMA per 6 sets alternating between the Sync HWDGE
  queue and GPSIMD SWDGE (keeping the ACT queue free for extraction).
  Host: numpy as_strided performs the diagonal shear
  out[yi,xi,dyi,dxi] = P[yi,xi,yi+dyi,xi+dxi] for free.
  in1 is pre-scaled by 2^-8 (exact in bf16) so no 1/C scale is needed.
"""
import sys

sys.path.insert(0, "/opt/trn_rl_repo")

import numpy as np

N_CORES = 8
B_LOC = 2          # batch elements per core
C, H, W = 256, 48, 64
ND = 21            # displacements per axis
NT, NU = 28, 36    # full t/u band dims per set (host-side box)
NUV = 26           # valid u columns per set after x-pad clipping
# per-g valid t windows (y' in [0,48)): g0 [10,28), g1 [2,26), g2 [0,18)
TCLIP = {0: (10, 28), 1: (2, 26), 2: (0, 18)}
# per-xw valid absolute u windows (xp in-bounds): xw0 [10,36), xw1 [16,42)
UCLIP = {0: (10, 36), 1: (16, 42)}
SL_SIZE = [468, 468, 624, 624, 468, 468]       # (th-tl)*NUV per sl=(g*2+xw)
SL_OFF = [0, 468, 936, 1560, 2184, 2652]
FB_ELEMS = 3120

_cache = {}


def _patch_ldw_opt():
    """Enable walrus LDWEIGHTS dedup: consecutive matmuls that reuse the
    same stationary operand (our two PSUM-bank chunks per cc) otherwise
    each pay a ~230 ns serialized drain->LDWEIGHTS->fill bubble."""
    import concourse.bass_utils as bu

    if getattr(bu.run_command, "_ldw_patched", False):
        return
    orig = bu.run_command

    def patched(argv, **kw):
        # ldw-opt=true rejected: "InstLdweights is not compatible with LDW
        # optimization" (CoreV3GenImpl.cpp:694) — keep args unchanged.
        return orig(argv, **kw)

    patched._ldw_patched = True
    bu.run_command = patched


def _build_module():
    import concourse.bacc as bacc
    import concourse.bass as bass
    import concourse.mybir as mybir
    import concourse.tile as tile

    f32 = mybir.dt.float32
    bf16 = mybir.dt.bfloat16
    f8 = mybir.dt.float8e4

    nc = bacc.Bacc(None, target_bir_lowering=False, debug=False)

    # in1s: [b, c, (yp,xpar,g,xw,yi,xi)] pre-scaled by 2^-8, bf16
    in1_d = nc.declare_dram_parameter("in1s", [B_LOC, C, 3072], bf16, isOutput=False)
    # in2s: [b, c, (q, h'24, xpar, u32)] unpadded (pads are never read)
    in2_d = nc.declare_dram_parameter("in2s", [B_LOC, C, 3072], bf16, isOutput=False)
    # o: [b, fb4, m(128), packed valid (t,u) spans of the 6 sl slots]
    o_d = nc.declare_dram_parameter(
        "o", [B_LOC, 4, 128, FB_ELEMS], bf16, isOutput=True
    )

    with tile.TileContext(nc) as tc:
        with (
            tc.tile_pool(name="inp", bufs=1) as inp,
            tc.tile_pool(name="dout", bufs=4) as dout,
            tc.tile_pool(name="ps", bufs=4, space=bass.MemorySpace.PSUM) as ps,
            tc.tile_pool(name="ps2", bufs=2, space=bass.MemorySpace.PSUM) as ps2,
        ):
            # HAM warmup: dummy matmuls on a zeroed tile while inputs load
            wz = inp.tile([128, 128], bf16, name="wz", tag="wz")
            Pd = ps.tile([128, 1, 512], f32, tag="P1")
            nc.vector.memset(wz[:], 0.0)
            for _ in range(44):
                nc.tensor.matmul(Pd[:, 0, 0:128], wz[:], wz[:], start=True, stop=True)

            a1 = {}
            a2 = {}
            for b in range(B_LOC):
                for cc in range(2):
                    a1[cc, b] = inp.tile(
                        [128, 3072], bf16, name=f"a1_{cc}{b}", tag=f"a1_{cc}{b}"
                    )
                    # [c, q(2), h'(24), xpar(2), u(32)]; u = (x - xpar)/2
                    a2[cc, b] = inp.tile(
                        [128, 2, 24, 2, 32], bf16, name=f"a2_{cc}{b}", tag=f"a2_{cc}{b}"
                    )
            # b0-fb0 inputs first so the first sets start as early as possible:
            # sync gets the first weight block halves, scalar the q0 bands.
            def load_a1(b, cc, lo, hi, eng):
                ch = slice(cc * 128, (cc + 1) * 128)
                eng.dma_start(a1[cc, b][:, lo:hi], in1_d[b, ch, lo:hi])

            def load_a2(b, cc, q, eng):
                ch = slice(cc * 128, (cc + 1) * 128)
                eng.dma_start(
                    a2[cc, b][:, q, :, :, :],
                    in2_d[b, ch, 1536 * q : 1536 * (q + 1)].rearrange(
                        "c (t x u) -> c t x u", t=24, x=2
                    ),
                )

            load_a1(0, 0, 0, 768, nc.sync)
            load_a1(0, 1, 0, 768, nc.sync)
            load_a2(0, 0, 0, nc.scalar)
            load_a2(0, 1, 0, nc.scalar)
            load_a1(0, 0, 768, 3072, nc.sync)
            load_a1(0, 1, 768, 3072, nc.sync)
            load_a2(0, 0, 1, nc.scalar)
            load_a2(0, 1, 1, nc.scalar)
            load_a1(1, 0, 0, 3072, nc.sync)
            load_a1(1, 1, 0, 3072, nc.sync)
            for cc in range(2):
                for q in range(2):
                    load_a2(1, cc, q, nc.scalar)

            for b in range(B_LOC):
                for fb in range(4):          # fb = yp*2 + xpar
                    yp, xpar = fb // 2, fb % 2
                    D = dout.tile(
                        [128, FB_ELEMS], bf16, name=f"D{b}{fb}", tag="D"
                    )
                    for sl in range(6):      # sl = g*2 + xw
                        g, xw = sl // 2, sl % 2
                        tl, th = TCLIP[g]
                        ulo, uhi = UCLIP[xw]
                        w_off = (fb * 6 + sl) * 128
                        # single-bank matmul when the whole valid band fits
                        # (g0/g2: 468 <= 512); two bank-chunks otherwise
                        if (th - tl) * NUV <= 512:
                            chunks = [(tl, th, 0)]
                            P = ps.tile([128, 1, 512], f32, tag="P1")
                        else:
                            chunks = [(tl, 14, 0), (14, th, 1)]
                            P = ps2.tile([128, 2, 512], f32, tag="P2")
                        for cc in range(2):
                            lhsT = a1[cc, b][:, w_off : w_off + 128]
                            for c_lo, c_hi, bank in chunks:
                                h0 = 8 * g + c_lo - 10
                                rhs = a2[cc, b][
                                    :, yp, h0 : h0 + (c_hi - c_lo), xpar,
                                    ulo - 10 : uhi - 10,
                                ]
                                nc.tensor.matmul(
                                    P[:, bank, 0 : (c_hi - c_lo) * NUV], lhsT, rhs,
                                    start=(cc == 0), stop=(cc == 1),
                                )
                        off = SL_OFF[sl]
                        eng_mul = (
                            nc.vector.tensor_scalar_mul if sl % 2 == 0
                            else nc.scalar.mul
                        )
                        pos = off
                        for c_lo, c_hi, bank in chunks:
                            sz = (c_hi - c_lo) * NUV
                            eng_mul(D[:, pos : pos + sz], P[:, bank, 0:sz], 1.0 / C)
                            pos += sz
                        if sl == 2:
                            nc.gpsimd.dma_start(o_d[b, fb, :, 0:1560], D[:, 0:1560])
                        elif sl == 4 and b == 1 and fb == 3:
                            nc.scalar.dma_start(
                                o_d[b, fb, :, 1560:2652], D[:, 1560:2652]
                            )
                        elif sl == 5:
                            if b == 1 and fb == 3:
                                nc.scalar.dma_start(
                                    o_d[b, fb, :, 2652:FB_ELEMS], D[:, 2652:FB_ELEMS]
                                )
                            else:
                                nc.scalar.dma_start(
                                    o_d[b, fb, :, 1560:FB_ELEMS], D[:, 1560:FB_ELEMS]
                                )
    nc.compile()
    return nc


def get_module():
    if "nc" not in _cache:
        _cache["nc"] = _build_module()
    return _cache["nc"]


def _prep_inputs(input1, input2):
    import ml_dtypes

    bf = ml_dtypes.bfloat16  # noqa: F841
    # in1: y(48)=(g3, yi8, yp2), x(64)=(xw2, xi16, xpar2)
    v1 = input1.astype(np.float32).reshape(16, C, 3, 8, 2, 2, 16, 2)
    in1s = np.ascontiguousarray(
        v1.transpose(0, 1, 4, 7, 2, 5, 3, 6).reshape(16, C, 3072)
    ).astype(bf)
    # in2: parity split only -- y(48) = (h'24, q2), x(64) = (u32, xpar2)
    v2 = input2.astype(np.float32).reshape(16, C, 24, 2, 32, 2)
    in2s = np.ascontiguousarray(
        v2.transpose(0, 1, 3, 2, 5, 4).reshape(16, C, 3072)
    ).astype(bf)
    return in1s, in2s


def _assemble(O):
    """O: [nb, 4fb, 128m, 4320] bf16 -> [nb, 441, 48, 64] f32."""
    nb = O.shape[0]
    # scatter packed valid spans into a zero-filled full [.., 3g, 2xw, 28t, 36u];
    # the zeros are exactly the out-of-range dy/dx displacements
    full = np.zeros((nb, 4, 128, 3, 2, NT, NU), O.dtype)
    for sl in range(6):
        g, xw = sl // 2, sl % 2
        tl, th = TCLIP[g]
        ulo, uhi = UCLIP[xw]
        full[:, :, :, g, xw, tl:th, ulo - 16 * xw : uhi - 16 * xw] = O[
            :, :, :, SL_OFF[sl] : SL_OFF[sl] + SL_SIZE[sl]
        ].reshape(nb, 4, 128, th - tl, NUV)
    # [b, yp, xpar, yi, xi, g, xw, t, u]
    V = full.reshape(nb, 2, 2, 8, 16, 3, 2, NT, NU)
    st = V.strides
    G = np.lib.stride_tricks.as_strided(
        V,
        shape=(nb, 2, 2, 3, 2, 8, 16, ND, ND),
        strides=(
            st[0], st[1], st[2], st[5], st[6],
            st[3] + st[7], st[4] + st[8], st[7], st[8],
        ),
    )
    # -> [b, dyi, dxi, g, yi, yp, xw, xi, xpar]
    out = G.transpose(0, 7, 8, 3, 5, 1, 4, 6, 2).astype(np.float32)
    return out.reshape(nb, ND * ND, H, W)


def kernel(input1: np.ndarray, input2: np.ndarray, _trace=False) -> np.ndarray:
    from concourse.bass_utils import run_bass_kernel_spmd

    _patch_ldw_opt()
    nc = get_module()
    in1s, in2s = _prep_inputs(input1, input2)
    in_maps = []
    for c in range(N_CORES):
        sl = slice(c * B_LOC, (c + 1) * B_LOC)
        in_maps.append({"in1s": in1s[sl], "in2s": in2s[sl]})
    res = run_bass_kernel_spmd(nc, in_maps, list(range(N_CORES)), trace=_trace)
    parts = [_assemble(res.results[c]["o"]) for c in range(N_CORES)]
    out = np.concatenate(parts, axis=0)
    if _trace:
        kernel.last_exec_time_ns = res.exec_time_ns
    return out


kernel.last_exec_time_ns = None


# revision 23
# speedup vs baseline: 1.0402x; 1.0402x over previous
"""FlowNetC correlation kernel for Trainium2 (8 NeuronCores, batch-sharded).

out[b, d, y, x] = mean_c in1[b,c,y,x] * in2pad[b,c, y+dy, x+dx],
d = dyi*21 + dxi, dy = 2*dyi-20, dx = 2*dxi-20  (441 displacements).

Design (per core, 2 batch elements):
  Parity decomposition: dy, dx are even, so (y,x) only pairs with (y',x')
  of equal parity. Weight sets = 128 in1 columns: 8 same-parity rows
  (yi) x 16 same-parity columns (xi), stored contiguous in SBUF so each
  LDWEIGHTS is one contiguous 128-wide load. Moving operand = in2 band
  [128c, t, 36 u']: same-parity rows y' = 16g+yp+2t-20 clipped to
  [0,48) and x-positions u' = 16*xw + xi + dxi (x padded to 104 on
  host). Out-of-range displacements become host-side zeros; no padding
  rows, no memsets. 4 matmuls per set (2 cc chunks x 2 PSUM banks,
  N <= 504), 192 total, full 128 PSUM partitions, measured warm at
  2.4 GHz (a dummy-matmul burst at t=0 pre-warms the HAM clock gate).
  Per set DVE extracts PSUM bank 0 and ACT bank 1 in parallel (valid
  rows only, cast bf16) into a compact [128, 4320] staging buffer; one
  contiguous ~1.1 MB DMA per 6 sets alternating between the Sync HWDGE
  queue and GPSIMD SWDGE (keeping the ACT queue free for extraction).
  Host: numpy as_strided performs the diagonal shear
  out[yi,xi,dyi,dxi] = P[yi,xi,yi+dyi,xi+dxi] for free.
  in1 is pre-scaled by 2^-8 (exact in bf16) so no 1/C scale is needed.
"""
import sys

sys.path.insert(0, "/opt/trn_rl_repo")

import numpy as np

N_CORES = 8
B_LOC = 2          # batch elements per core
C, H, W = 256, 48, 64
ND = 21            # displacements per axis
NT, NU = 28, 36    # full t/u band dims per set (host-side box)
NUV = 26           # valid u columns per set after x-pad clipping
# per-g valid t windows (y' in [0,48)): g0 [10,28), g1 [2,26), g2 [0,18)
TCLIP = {0: (10, 28), 1: (2, 26), 2: (0, 18)}
# per-xw valid absolute u windows (xp in-bounds): xw0 [10,36), xw1 [16,42)
UCLIP = {0: (10, 36), 1: (16, 42)}
SL_SIZE = [468, 468, 624, 624, 468, 468]       # (th-tl)*NUV per sl=(g*2+xw)
SL_OFF = [0, 468, 936, 1560, 2184, 2652]
FB_ELEMS = 3120

_cache = {}


def _patch_ldw_opt():
    """Enable walrus LDWEIGHTS dedup: consecutive matmuls that reuse the
    same stationary operand (our two PSUM-bank chunks per cc) otherwise
    each pay a ~230 ns serialized drain->LDWEIGHTS->fill bubble."""
    import concourse.bass_utils as bu

    if getattr(bu.run_command, "_ldw_patched", False):
        return
    orig = bu.run_command

    def patched(argv, **kw):
        # ldw-opt=true rejected: "InstLdweights is not compatible with LDW
        # optimization" (CoreV3GenImpl.cpp:694) — keep args unchanged.
        return orig(argv, **kw)

    patched._ldw_patched = True
    bu.run_command = patched


def _build_module():
    import concourse.bacc as bacc
    import concourse.bass as bass
    import concourse.mybir as mybir
    import concourse.tile as tile

    f32 = mybir.dt.float32
    bf16 = mybir.dt.bfloat16
    f8 = mybir.dt.float8e4

    nc = bacc.Bacc(None, target_bir_lowering=False, debug=False)

    # in1s: [b, c, (yp,xpar,g,xw,yi,xi)] pre-scaled by 2^-8, bf16
    in1_d = nc.declare_dram_parameter("in1s", [B_LOC, C, 3072], bf16, isOutput=False)
    # in2s: [b, c, (q, h'24, xpar, u32)] unpadded (pads are never read)
    in2_d = nc.declare_dram_parameter("in2s", [B_LOC, C, 3072], bf16, isOutput=False)
    # o: [b, fb4, m(128), packed valid (t,u) spans of the 6 sl slots]
    o_d = nc.declare_dram_parameter(
        "o", [B_LOC, 4, 128, FB_ELEMS], bf16, isOutput=True
    )

    with tile.TileContext(nc) as tc:
        with (
            tc.tile_pool(name="inp", bufs=1) as inp,
            tc.tile_pool(name="dout", bufs=4) as dout,
            tc.tile_pool(name="ps", bufs=4, space=bass.MemorySpace.PSUM) as ps,
        ):
            # HAM warmup: dummy matmuls on a zeroed tile while inputs load
            wz = inp.tile([128, 128], bf16, name="wz", tag="wz")
            Pd = ps.tile([128, 2, 512], f32, tag="P")
            nc.vector.memset(wz[:], 0.0)
            for _ in range(44):
                nc.tensor.matmul(Pd[:, 0, 0:128], wz[:], wz[:], start=True, stop=True)

            a1 = {}
            a2 = {}
            for b in range(B_LOC):
                for cc in range(2):
                    a1[cc, b] = inp.tile(
                        [128, 3072], bf16, name=f"a1_{cc}{b}", tag=f"a1_{cc}{b}"
                    )
                    # [c, q(2), h'(24), xpar(2), u(32)]; u = (x - xpar)/2
                    a2[cc, b] = inp.tile(
                        [128, 2, 24, 2, 32], bf16, name=f"a2_{cc}{b}", tag=f"a2_{cc}{b}"
                    )
            # b0-fb0 inputs first so the first sets start as early as possible:
            # sync gets the first weight block halves, scalar the q0 bands.
            def load_a1(b, cc, lo, hi, eng):
                ch = slice(cc * 128, (cc + 1) * 128)
                eng.dma_start(a1[cc, b][:, lo:hi], in1_d[b, ch, lo:hi])

            def load_a2(b, cc, q, eng):
                ch = slice(cc * 128, (cc + 1) * 128)
                eng.dma_start(
                    a2[cc, b][:, q, :, :, :],
                    in2_d[b, ch, 1536 * q : 1536 * (q + 1)].rearrange(
                        "c (t x u) -> c t x u", t=24, x=2
                    ),
                )

            load_a1(0, 0, 0, 768, nc.sync)
            load_a1(0, 1, 0, 768, nc.sync)
            load_a2(0, 0, 0, nc.scalar)
            load_a2(0, 1, 0, nc.scalar)
            load_a1(0, 0, 768, 3072, nc.sync)
            load_a1(0, 1, 768, 3072, nc.sync)
            load_a2(0, 0, 1, nc.scalar)
            load_a2(0, 1, 1, nc.scalar)
            load_a1(1, 0, 0, 3072, nc.sync)
            load_a1(1, 1, 0, 3072, nc.sync)
            for cc in range(2):
                for q in range(2):
                    load_a2(1, cc, q, nc.scalar)

            for b in range(B_LOC):
                for fb in range(4):          # fb = yp*2 + xpar
                    yp, xpar = fb // 2, fb % 2
                    D = dout.tile(
                        [128, FB_ELEMS], bf16, name=f"D{b}{fb}", tag="D"
                    )
                    for sl in range(6):      # sl = g*2 + xw
                        g, xw = sl // 2, sl % 2
                        tl, th = TCLIP[g]
                        ulo, uhi = UCLIP[xw]
                        P = ps.tile([128, 2, 512], f32, tag="P")
                        w_off = (fb * 6 + sl) * 128
                        # single-bank matmul when the whole valid band fits
                        # (g0/g2: 468 <= 512); two bank-chunks otherwise
                        if (th - tl) * NUV <= 512:
                            chunks = [(tl, th, 0)]
                        else:
                            chunks = [(tl, 14, 0), (14, th, 1)]
                        for cc in range(2):
                            lhsT = a1[cc, b][:, w_off : w_off + 128]
                            for c_lo, c_hi, bank in chunks:
                                h0 = 8 * g + c_lo - 10
                                rhs = a2[cc, b][
                                    :, yp, h0 : h0 + (c_hi - c_lo), xpar,
                                    ulo - 10 : uhi - 10,
                                ]
                                nc.tensor.matmul(
                                    P[:, bank, 0 : (c_hi - c_lo) * NUV], lhsT, rhs,
                                    start=(cc == 0), stop=(cc == 1),
                                )
                        off = SL_OFF[sl]
                        eng_mul = (
                            nc.vector.tensor_scalar_mul if sl % 2 == 0
                            else nc.scalar.mul
                        )
                        pos = off
                        for c_lo, c_hi, bank in chunks:
                            sz = (c_hi - c_lo) * NUV
                            eng_mul(D[:, pos : pos + sz], P[:, bank, 0:sz], 1.0 / C)
                            pos += sz
                        if sl == 2:
                            nc.gpsimd.dma_start(o_d[b, fb, :, 0:1560], D[:, 0:1560])
                        elif sl == 4 and b == 1 and fb == 3:
                            nc.scalar.dma_start(
                                o_d[b, fb, :, 1560:2652], D[:, 1560:2652]
                            )
                        elif sl == 5:
                            if b == 1 and fb == 3:
                                nc.scalar.dma_start(
                                    o_d[b, fb, :, 2652:FB_ELEMS], D[:, 2652:FB_ELEMS]
                                )
                            else:
                                nc.scalar.dma_start(
                                    o_d[b, fb, :, 1560:FB_ELEMS], D[:, 1560:FB_ELEMS]
                                )
    nc.compile()
    return nc


def get_module():
    if "nc" not in _cache:
        _cache["nc"] = _build_module()
    return _cache["nc"]


def _prep_inputs(input1, input2):
    import ml_dtypes

    bf = ml_dtypes.bfloat16  # noqa: F841
    # in1: y(48)=(g3, yi8, yp2), x(64)=(xw2, xi16, xpar2)
    v1 = input1.astype(np.float32).reshape(16, C, 3, 8, 2, 2, 16, 2)
    in1s = np.ascontiguousarray(
        v1.transpose(0, 1, 4, 7, 2, 5, 3, 6).reshape(16, C, 3072)
    ).astype(bf)
    # in2: parity split only -- y(48) = (h'24, q2), x(64) = (u32, xpar2)
    v2 = input2.astype(np.float32).reshape(16, C, 24, 2, 32, 2)
    in2s = np.ascontiguousarray(
        v2.transpose(0, 1, 3, 2, 5, 4).reshape(16, C, 3072)
    ).astype(bf)
    return in1s, in2s


def _assemble(O):
    """O: [nb, 4fb, 128m, 4320] bf16 -> [nb, 441, 48, 64] f32."""
    nb = O.shape[0]
    # scatter packed valid spans into a zero-filled full [.., 3g, 2xw, 28t, 36u];
    # the zeros are exactly the out-of-range dy/dx displacements
    full = np.zeros((nb, 4, 128, 3, 2, NT, NU), O.dtype)
    for sl in range(6):
        g, xw = sl // 2, sl % 2
        tl, th = TCLIP[g]
        ulo, uhi = UCLIP[xw]
        full[:, :, :, g, xw, tl:th, ulo - 16 * xw : uhi - 16 * xw] = O[
            :, :, :, SL_OFF[sl] : SL_OFF[sl] + SL_SIZE[sl]
        ].reshape(nb, 4, 128, th - tl, NUV)
    # [b, yp, xpar, yi, xi, g, xw, t, u]
    V = full.reshape(nb, 2, 2, 8, 16, 3, 2, NT, NU)
    st = V.strides
    G = np.lib.stride_tricks.as_strided(
        V,
        shape=(nb, 2, 2, 3, 2, 8, 16, ND, ND),
        strides=(
            st[0], st[1], st[2], st[5], st[6],
            st[3] + st[7], st[4] + st[8], st[7], st[8],
        ),
    )
    # -> [b, dyi, dxi, g, yi, yp, xw, xi, xpar]
    out = G.transpose(0, 7, 8, 3, 5, 1, 4, 6, 2).astype(np.float32)
    return out.reshape(nb, ND * ND, H, W)


def kernel(input1: np.ndarray, input2: np.ndarray, _trace=False) -> np.ndarray:
    from concourse.bass_utils import run_bass_kernel_spmd

    _patch_ldw_opt()
    nc = get_module()
    in1s, in2s = _prep_inputs(input1, input2)
    in_maps = []
    for c in range(N_CORES):
        sl = slice(c * B_LOC, (c + 1) * B_LOC)
        in_maps.append({"in1s": in1s[sl], "in2s": in2s[sl]})
    res = run_bass_kernel_spmd(nc, in_maps, list(range(N_CORES)), trace=_trace)
    parts = [_assemble(res.results[c]["o"]) for c in range(N_CORES)]
    out = np.concatenate(parts, axis=0)
    if _trace:
        kernel.last_exec_time_ns = res.exec_time_ns
    return out


kernel.last_exec_time_ns = None


# revision 24
# speedup vs baseline: 1.0892x; 1.0472x over previous
"""FlowNetC correlation kernel for Trainium2 (8 NeuronCores, batch-sharded).

out[b, d, y, x] = mean_c in1[b,c,y,x] * in2pad[b,c, y+dy, x+dx],
d = dyi*21 + dxi, dy = 2*dyi-20, dx = 2*dxi-20  (441 displacements).

Design (per core, 2 batch elements):
  Parity decomposition: dy, dx are even, so (y,x) only pairs with (y',x')
  of equal parity. Weight sets = 128 in1 columns: 8 same-parity rows
  (yi) x 16 same-parity columns (xi), stored contiguous in SBUF so each
  LDWEIGHTS is one contiguous 128-wide load. Moving operand = in2 band
  [128c, t, 36 u']: same-parity rows y' = 16g+yp+2t-20 clipped to
  [0,48) and x-positions u' = 16*xw + xi + dxi (x padded to 104 on
  host). Out-of-range displacements become host-side zeros; no padding
  rows, no memsets. 4 matmuls per set (2 cc chunks x 2 PSUM banks,
  N <= 504), 192 total, full 128 PSUM partitions, measured warm at
  2.4 GHz (a dummy-matmul burst at t=0 pre-warms the HAM clock gate).
  Per set DVE extracts PSUM bank 0 and ACT bank 1 in parallel (valid
  rows only, cast bf16) into a compact [128, 4320] staging buffer; one
  contiguous ~1.1 MB DMA per 6 sets alternating between the Sync HWDGE
  queue and GPSIMD SWDGE (keeping the ACT queue free for extraction).
  Host: numpy as_strided performs the diagonal shear
  out[yi,xi,dyi,dxi] = P[yi,xi,yi+dyi,xi+dxi] for free.
  in1 is pre-scaled by 2^-8 (exact in bf16) so no 1/C scale is needed.
"""
import sys

sys.path.insert(0, "/opt/trn_rl_repo")

import numpy as np

N_CORES = 8
B_LOC = 2          # batch elements per core
C, H, W = 256, 48, 64
ND = 21            # displacements per axis
NT, NU = 28, 36    # full t/u band dims per set (host-side box)
NUV = 26           # valid u columns per set after x-pad clipping
# per-g valid t windows (y' in [0,48)): g0 [10,28), g1 [2,26), g2 [0,18)
TCLIP = {0: (10, 28), 1: (2, 26), 2: (0, 18)}
# per-xw valid absolute u windows (xp in-bounds): xw0 [10,36), xw1 [16,42)
UCLIP = {0: (10, 36), 1: (16, 42)}
SL_SIZE = [468, 468, 624, 624, 468, 468]       # (th-tl)*NUV per sl=(g*2+xw)
SL_OFF = [0, 468, 936, 1560, 2184, 2652]
FB_ELEMS = 3120

_cache = {}


def _patch_ldw_opt():
    """Enable walrus LDWEIGHTS dedup: consecutive matmuls that reuse the
    same stationary operand (our two PSUM-bank chunks per cc) otherwise
    each pay a ~230 ns serialized drain->LDWEIGHTS->fill bubble."""
    import concourse.bass_utils as bu

    if getattr(bu.run_command, "_ldw_patched", False):
        return
    orig = bu.run_command

    def patched(argv, **kw):
        # ldw-opt=true rejected: "InstLdweights is not compatible with LDW
        # optimization" (CoreV3GenImpl.cpp:694) — keep args unchanged.
        return orig(argv, **kw)

    patched._ldw_patched = True
    bu.run_command = patched


def _build_module():
    import concourse.bacc as bacc
    import concourse.bass as bass
    import concourse.mybir as mybir
    import concourse.tile as tile

    f32 = mybir.dt.float32
    bf16 = mybir.dt.bfloat16
    f8 = mybir.dt.float8e4

    nc = bacc.Bacc(None, target_bir_lowering=False, debug=False)

    # in1s: [b, c, (yp,xpar,g,xw,yi,xi)] pre-scaled by 2^-8, bf16
    in1_d = nc.declare_dram_parameter("in1s", [B_LOC, C, 3072], bf16, isOutput=False)
    # in2s: [b, c, (q, h'24, xpar, u32)] unpadded (pads are never read)
    in2_d = nc.declare_dram_parameter("in2s", [B_LOC, C, 3072], bf16, isOutput=False)
    # o: [b, fb4, m(128), packed valid (t,u) spans of the 6 sl slots]
    o_d = nc.declare_dram_parameter(
        "o", [B_LOC, 4, 128, FB_ELEMS], bf16, isOutput=True
    )

    with tile.TileContext(nc) as tc:
        with (
            tc.tile_pool(name="inp", bufs=1) as inp,
            tc.tile_pool(name="dout", bufs=4) as dout,
            tc.tile_pool(name="ps", bufs=4, space=bass.MemorySpace.PSUM) as ps,
        ):
            # HAM warmup: dummy matmuls on a zeroed tile while inputs load
            wz = inp.tile([128, 128], bf16, name="wz", tag="wz")
            Pd = ps.tile([128, 2, 512], f32, tag="P")
            nc.vector.memset(wz[:], 0.0)
            for _ in range(44):
                nc.tensor.matmul(Pd[:, 0, 0:128], wz[:], wz[:], start=True, stop=True)

            a1 = {}
            a2 = {}
            for b in range(B_LOC):
                for cc in range(2):
                    a1[cc, b] = inp.tile(
                        [128, 3072], bf16, name=f"a1_{cc}{b}", tag=f"a1_{cc}{b}"
                    )
                    # [c, q(2), h'(24), xpar(2), u(32)]; u = (x - xpar)/2
                    a2[cc, b] = inp.tile(
                        [128, 2, 24, 2, 32], bf16, name=f"a2_{cc}{b}", tag=f"a2_{cc}{b}"
                    )
            # b0-fb0 inputs first so the first sets start as early as possible:
            # sync gets the first weight block halves, scalar the q0 bands.
            def load_a1(b, cc, lo, hi, eng):
                ch = slice(cc * 128, (cc + 1) * 128)
                eng.dma_start(a1[cc, b][:, lo:hi], in1_d[b, ch, lo:hi])

            def load_a2(b, cc, q, eng):
                ch = slice(cc * 128, (cc + 1) * 128)
                eng.dma_start(
                    a2[cc, b][:, q, :, :, :],
                    in2_d[b, ch, 1536 * q : 1536 * (q + 1)].rearrange(
                        "c (t x u) -> c t x u", t=24, x=2
                    ),
                )

            load_a1(0, 0, 0, 768, nc.sync)
            load_a1(0, 1, 0, 768, nc.sync)
            load_a2(0, 0, 0, nc.scalar)
            load_a2(0, 1, 0, nc.scalar)
            load_a1(1, 0, 0, 768, nc.sync)
            load_a1(1, 1, 0, 768, nc.sync)
            load_a2(1, 0, 0, nc.scalar)
            load_a2(1, 1, 0, nc.scalar)
            load_a1(0, 0, 768, 3072, nc.sync)
            load_a1(0, 1, 768, 3072, nc.sync)
            load_a2(0, 0, 1, nc.scalar)
            load_a2(0, 1, 1, nc.scalar)
            load_a1(1, 0, 768, 3072, nc.sync)
            load_a1(1, 1, 768, 3072, nc.sync)
            load_a2(1, 0, 1, nc.scalar)
            load_a2(1, 1, 1, nc.scalar)

            for b in range(B_LOC):
                for fb in range(4):          # fb = yp*2 + xpar
                    yp, xpar = fb // 2, fb % 2
                    D = dout.tile(
                        [128, FB_ELEMS], bf16, name=f"D{b}{fb}", tag="D"
                    )
                    for sl in range(6):      # sl = g*2 + xw
                        g, xw = sl // 2, sl % 2
                        tl, th = TCLIP[g]
                        ulo, uhi = UCLIP[xw]
                        P = ps.tile([128, 2, 512], f32, tag="P")
                        w_off = (fb * 6 + sl) * 128
                        # single-bank matmul when the whole valid band fits
                        # (g0/g2: 468 <= 512); two bank-chunks otherwise
                        if (th - tl) * NUV <= 512:
                            chunks = [(tl, th, 0)]
                        else:
                            chunks = [(tl, 14, 0), (14, th, 1)]
                        for cc in range(2):
                            lhsT = a1[cc, b][:, w_off : w_off + 128]
                            for c_lo, c_hi, bank in chunks:
                                h0 = 8 * g + c_lo - 10
                                rhs = a2[cc, b][
                                    :, yp, h0 : h0 + (c_hi - c_lo), xpar,
                                    ulo - 10 : uhi - 10,
                                ]
                                nc.tensor.matmul(
                                    P[:, bank, 0 : (c_hi - c_lo) * NUV], lhsT, rhs,
                                    start=(cc == 0), stop=(cc == 1),
                                )
                        off = SL_OFF[sl]
                        eng_mul = (
                            nc.vector.tensor_scalar_mul if sl % 2 == 0
                            else nc.scalar.mul
                        )
                        pos = off
                        for c_lo, c_hi, bank in chunks:
                            sz = (c_hi - c_lo) * NUV
                            eng_mul(D[:, pos : pos + sz], P[:, bank, 0:sz], 1.0 / C)
                            pos += sz
                        if sl == 2:
                            nc.gpsimd.dma_start(o_d[b, fb, :, 0:1560], D[:, 0:1560])
                        elif sl == 4 and b == 1 and fb == 3:
                            nc.scalar.dma_start(
                                o_d[b, fb, :, 1560:2652], D[:, 1560:2652]
                            )
                        elif sl == 5:
                            if b == 1 and fb == 3:
                                nc.scalar.dma_start(
                                    o_d[b, fb, :, 2652:FB_ELEMS], D[:, 2652:FB_ELEMS]
                                )
                            else:
                                nc.scalar.dma_start(
                                    o_d[b, fb, :, 1560:FB_ELEMS], D[:, 1560:FB_ELEMS]
                                )
    nc.compile()
    return nc


def get_module():
    if "nc" not in _cache:
        _cache["nc"] = _build_module()
    return _cache["nc"]


def _prep_inputs(input1, input2):
    import ml_dtypes

    bf = ml_dtypes.bfloat16  # noqa: F841
    # in1: y(48)=(g3, yi8, yp2), x(64)=(xw2, xi16, xpar2)
    v1 = input1.astype(np.float32).reshape(16, C, 3, 8, 2, 2, 16, 2)
    in1s = np.ascontiguousarray(
        v1.transpose(0, 1, 4, 7, 2, 5, 3, 6).reshape(16, C, 3072)
    ).astype(bf)
    # in2: parity split only -- y(48) = (h'24, q2), x(64) = (u32, xpar2)
    v2 = input2.astype(np.float32).reshape(16, C, 24, 2, 32, 2)
    in2s = np.ascontiguousarray(
        v2.transpose(0, 1, 3, 2, 5, 4).reshape(16, C, 3072)
    ).astype(bf)
    return in1s, in2s


def _assemble(O):
    """O: [nb, 4fb, 128m, 4320] bf16 -> [nb, 441, 48, 64] f32."""
    nb = O.shape[0]
    # scatter packed valid spans into a zero-filled full [.., 3g, 2xw, 28t, 36u];
    # the zeros are exactly the out-of-range dy/dx displacements
    full = np.zeros((nb, 4, 128, 3, 2, NT, NU), O.dtype)
    for sl in range(6):
        g, xw = sl // 2, sl % 2
        tl, th = TCLIP[g]
        ulo, uhi = UCLIP[xw]
        full[:, :, :, g, xw, tl:th, ulo - 16 * xw : uhi - 16 * xw] = O[
            :, :, :, SL_OFF[sl] : SL_OFF[sl] + SL_SIZE[sl]
        ].reshape(nb, 4, 128, th - tl, NUV)
    # [b, yp, xpar, yi, xi, g, xw, t, u]
    V = full.reshape(nb, 2, 2, 8, 16, 3, 2, NT, NU)
    st = V.strides
    G = np.lib.stride_tricks.as_strided(
        V,
        shape=(nb, 2, 2, 3, 2, 8, 16, ND, ND),
        strides=(
            st[0], st[1], st[2], st[5], st[6],
            st[3] + st[7], st[4] + st[8], st[7], st[8],
        ),
    )
    # -> [b, dyi, dxi, g, yi, yp, xw, xi, xpar]
    out = G.transpose(0, 7, 8, 3, 5, 1, 4, 6, 2).astype(np.float32)
    return out.reshape(nb, ND * ND, H, W)


def kernel(input1: np.ndarray, input2: np.ndarray, _trace=False) -> np.ndarray:
    from concourse.bass_utils import run_bass_kernel_spmd

    _patch_ldw_opt()
    nc = get_module()
    in1s, in2s = _prep_inputs(input1, input2)
    in_maps = []
    for c in range(N_CORES):
        sl = slice(c * B_LOC, (c + 1) * B_LOC)
        in_maps.append({"in1s": in1s[sl], "in2s": in2s[sl]})
    res = run_bass_kernel_spmd(nc, in_maps, list(range(N_CORES)), trace=_trace)
    parts = [_assemble(res.results[c]["o"]) for c in range(N_CORES)]
    out = np.concatenate(parts, axis=0)
    if _trace:
        kernel.last_exec_time_ns = res.exec_time_ns
    return out


kernel.last_exec_time_ns = None
